# revision 16
# baseline (speedup 1.0000x reference)
"""Trainium2 Bass kernel for the GNN point-cloud encoder (KNN graph conv x3 + 2 poolings).

Data parallel over batch: 8 batch elements -> 8 NeuronCores, one full encoder per core.
Accepts FULL inputs, returns FULL outputs (v2, fm3) as numpy arrays.
"""

import contextlib
import sys

sys.path.insert(0, "/opt/trn_rl_repo")

import numpy as np

import concourse.bass as bass
import concourse.mybir as mybir
import concourse.tile as tile
from concourse import bacc

F32 = mybir.dt.float32
F16 = mybir.dt.float16
U16 = mybir.dt.uint16
I16 = mybir.dt.int16
I32 = mybir.dt.int32
ALU = mybir.AluOpType
ACTF = mybir.ActivationFunctionType
AXX = mybir.AxisListType.X
IOA = bass.IndirectOffsetOnAxis

# Problem constants (hardcoded per the harness contract)
BS = 8
N = 2048
NB = 20          # neighbor count for conv layers
SUP = 7          # support_num
K0, C1, C2 = 32, 64, 256
N1 = 256         # vertices after pooling 1 (2048 / 8)
N2 = 16          # vertices after pooling 2 (256 / 16)
T16 = N // 128   # 16 row tiles
NEG = -3.0e38

SK0 = SUP * K0            # 224
SC1 = SUP * C1            # 448
SC2 = SUP * C2            # 1792
W01 = SK0 + SC1           # 672

SAMPLE1 = np.random.default_rng(1).permutation(N)[:N1]          # pooling 1 sample
SAMPLE2 = np.random.default_rng(2).permutation(N1)[:N2]         # pooling 2 sample


def _ap(t, dims, off=0):
    """Raw access pattern on the tensor behind AP `t` (DRAM only; steps in elements)."""
    return bass.AP(tensor=t.tensor, offset=t.offset + off, ap=[list(d) for d in dims])


def build_nc():
    nc = bacc.Bacc("TRN2", target_bir_lowering=False, debug=False)

    verts = nc.dram_tensor("vertices", [N, 3], F32, kind="ExternalInput")
    w01_disp = nc.dram_tensor("w01_disp", [3, W01], F16, kind="ExternalInput")
    w2_disp = nc.dram_tensor("w2_disp", [3, SC2], F16, kind="ExternalInput")
    w0w = nc.dram_tensor("w0w", [128, SK0], F32, kind="ExternalInput")
    wb1 = nc.dram_tensor("wb1", [K0 + 1, (SUP + 1) * C1], F32, kind="ExternalInput")
    wb2 = nc.dram_tensor("wb2", [C1 + 1, (SUP + 1) * C2], F32, kind="ExternalInput")
    neg_i = nc.dram_tensor("neg_i", [128, 128], F32, kind="ExternalInput")
    identity = nc.dram_tensor("identity", [128, 128], F32, kind="ExternalInput")
    samp1_pp = nc.dram_tensor("samp1_pp", [128, 2], I32, kind="ExternalInput")
    samp2_pp = nc.dram_tensor("samp2_pp", [16, 1], I32, kind="ExternalInput")
    ones_row = nc.dram_tensor("ones_row", [1, N], F32, kind="ExternalInput")

    out_v2 = nc.dram_tensor("out_v2", [N2, 3], F32, kind="ExternalOutput")
    out_fm3 = nc.dram_tensor("out_fm3", [N2, C2], F32, kind="ExternalOutput")

    with tile.TileContext(nc) as tc:
        with contextlib.ExitStack() as ctx:
            dram = ctx.enter_context(tc.tile_pool(name="dram", bufs=1, space="DRAM"))
            const = ctx.enter_context(tc.tile_pool(name="const", bufs=1))
            pers = ctx.enter_context(tc.tile_pool(name="pers", bufs=1))
            ctx_a = ctx.enter_context(contextlib.ExitStack())

            # ---- DRAM scratch tables ----
            T0 = dram.tile([N, 64], F32, name="T0", uniquify=False)          # padded coords
            T1 = dram.tile([N, 512], F16, name="T1", uniquify=False)         # feat1 support
            T2 = dram.tile([N, 128], F16, name="T2", uniquify=False)         # fm1 padded
            T3 = dram.tile([N1, SC2], F16, name="T3", uniquify=False)        # feat2 support
            T4 = dram.tile([N1, C2], F16, name="T4", uniquify=False)         # fm2
            TNI = dram.tile([N, 128], I16, name="TNI", uniquify=False)       # NI padded rows
            TNI2 = dram.tile([N1, 128], I16, name="TNI2", uniquify=False)
            T0v1 = dram.tile([N1, 64], F32, name="T0v1", uniquify=False)     # pooled coords
            TdT = dram.tile([T16, 3, NB, 128], F16, name="TdT", uniquify=False)
            FB = dram.tile([T16 * 2560], I16, name="FB", uniquify=False)     # q-major lists
            FBT = dram.tile([T16 * 2560], I16, name="FBT", uniquify=False)   # wrapped
            FC = dram.tile([2 * 2560], I16, name="FC", uniquify=False)       # knn2 lists
            FCT = dram.tile([2 * 2560], I16, name="FCT", uniquify=False)

            # ---- constants / weights to SBUF ----
            neg_i_sb = const.tile([128, 128], F32)
            nc.sync.dma_start(neg_i_sb, neg_i[:, :])
            ident_sb = const.tile([128, 128], F32)
            nc.sync.dma_start(ident_sb, identity[:, :])
            w01_sb = const.tile([3, W01], F16)
            nc.sync.dma_start(w01_sb, w01_disp[:, :])
            w2d_sb = const.tile([3, SC2], F16)
            nc.sync.dma_start(w2d_sb, w2_disp[:, :])
            w0w_sb = const.tile([128, SK0], F32)
            nc.sync.dma_start(w0w_sb, w0w[:, :])
            wb1_sb = const.tile([K0 + 1, (SUP + 1) * C1], F32)
            nc.sync.dma_start(wb1_sb, wb1[:, :])
            wb2_sb = const.tile([C1 + 1, (SUP + 1) * C2], F32)
            nc.sync.dma_start(wb2_sb, wb2[:, :])
            s1pp_sb = const.tile([128, 2], I32)
            nc.sync.dma_start(s1pp_sb, samp1_pp[:, :])
            s2pp_sb = const.tile([16, 1], I32)
            nc.sync.dma_start(s2pp_sb, samp2_pp[:, :])

            # per-layer state: pers_a dies after conv1, pers stays to the end
            pers_a = ctx_a.enter_context(tc.tile_pool(name="pers_a", bufs=1))
            NI = pers_a.tile([128, T16, 24], U16)
            NI32 = pers_a.tile([128, T16, NB], I32)
            V_sb = pers_a.tile([128, T16, 3], F32)     # vertex-major
            nc.sync.dma_start(V_sb, _ap(verts[:, :], [[3, 128], [384, T16], [1, 3]]))
            VT16 = pers_a.tile([16, N], F32)           # rows 0:3 = V.T, rest 0
            nc.vector.memset(VT16, 0.0)
            nc.sync.dma_start(VT16[0:3, :], _ap(verts[:, :], [[1, 3], [3, N]]))
            idxw = pers_a.tile([16, T16, 160], I16)    # wrap-16 q-lists per tile
            f1c = pers_a.tile([128, T16, C1], F32)
            fm01T = pers_a.tile([K0 + 1, T16, 128], F32)
            f2T = pers.tile([C1 + 1, 2, 128], F32)
            v1g = pers.tile([128, 2, 64], F32)
            NI2 = pers.tile([128, 2, 24], U16)
            NI232 = pers.tile([128, 2, NB], I32)
            idxw2 = pers.tile([16, 2, 160], I16)

            # ---- Phase 0 + 1: prep and KNN over 2048 vertices ----
            with (
                tc.tile_pool(name="knn_psum", bufs=2, space="PSUM") as kpsum,
                tc.tile_pool(name="knn_sb", bufs=2) as ksb,
                tc.tile_pool(name="prep_sb", bufs=1) as psb,
            ):
                lhsT4 = psb.tile([4, N], F32)
                nc.sync.dma_start(lhsT4[0:3, :], _ap(verts[:, :], [[1, 3], [3, N]]))
                nc.sync.dma_start(lhsT4[3:4, :], ones_row[:, :])
                rhs4 = psb.tile([4, N], F32)
                nc.vector.tensor_scalar_mul(rhs4[0:3, :], lhsT4[0:3, :], 2.0)

                # q_j = |v_j|^2 -> rhs4 row 3 as -q
                vsq = psb.tile([128, T16, 3], F32)
                nc.vector.tensor_mul(vsq, V_sb, V_sb)
                q_vm = psb.tile([128, T16], F32)
                nc.vector.tensor_reduce(q_vm, vsq, axis=AXX, op=ALU.add)
                qT_ps = kpsum.tile([T16, 128], F32, tag="m_ps")
                nc.tensor.transpose(qT_ps, q_vm, ident_sb)
                qT_sb = psb.tile([T16, 128], F32)
                nc.scalar.activation(qT_sb, qT_ps, ACTF.Copy, scale=-1.0)
                nc.sync.dma_start(
                    rhs4[3:4, :].rearrange("a (t p) -> a t p", p=128), qT_sb
                )

                # zero T0 padding then write coords
                zpad = psb.tile([128, 1024], F32)
                nc.vector.memset(zpad, 0.0)
                nc.sync.dma_start(_ap(T0, [[1024, 128], [1, 1024]]), zpad)
                nc.sync.dma_start(_ap(T0, [[64, 128], [8192, T16], [1, 3]]), V_sb)

                for t in range(T16):
                    m_ps = kpsum.tile([128, 4, 512], F32, tag="m_ps")
                    for j in range(4):
                        nc.tensor.matmul(
                            m_ps[:, j, :],
                            lhsT4[:, t * 128:(t + 1) * 128],
                            rhs4[:, j * 512:(j + 1) * 512],
                            start=True, stop=True,
                        )
                    m_sb = ksb.tile([128, N], F32, tag="m_sb")
                    nc.scalar.activation(
                        m_sb, m_ps.rearrange("p a b -> p (a b)"), ACTF.Copy
                    )
                    nc.vector.tensor_add(
                        m_sb[:, t * 128:(t + 1) * 128],
                        m_sb[:, t * 128:(t + 1) * 128],
                        neg_i_sb,
                    )
                    for r in range(3):
                        rv = ksb.tile([128, 8], F32, tag="rv")
                        nc.vector.max(out=rv, in_=m_sb)
                        nc.vector.max_index(
                            out=NI[:, t, r * 8:(r + 1) * 8], in_max=rv, in_values=m_sb
                        )
                        if r < 2:
                            nc.vector.match_replace(
                                out=m_sb, in_to_replace=rv, in_values=m_sb,
                                imm_value=NEG,
                            )

            nc.vector.tensor_copy(NI32, NI[:, :, 0:NB])
            # NI -> TNI (padded rows, for pooling-1 sampled lookup)
            nc.sync.dma_start(
                _ap(TNI, [[128, 128], [16384, T16], [1, 24]]),
                NI[:, :, :].bitcast(I16),
            )
            # wrap-16 q-ordered index lists for ap_gather (via elementwise DMAs)
            for t in range(T16):
                nc.sync.dma_start(
                    _ap(FB, [[1, 128], [128, NB], [1, 1]], off=t * 2560),
                    NI[:, t:t + 1, 0:NB].bitcast(I16).rearrange("p a n -> p n a"),
                )
                nc.sync.dma_start(
                    _ap(FBT, [[160, 16], [1, 160], [1, 1]], off=t * 2560),
                    _ap(FB, [[1, 16], [16, 160], [1, 1]], off=t * 2560),
                )
            nc.sync.dma_start(idxw, _ap(FBT, [[160, 16], [2560, T16], [1, 160]]))

            # ---- Phase 2: conv0 (layer 0) + feat1, per tile ----
            nc.vector.memset(fm01T[K0:K0 + 1, :, :], 1.0)
            with (
                tc.tile_pool(name="l0_psum", bufs=3, space="PSUM") as l0ps,
                tc.tile_pool(name="l0_sb", bufs=2) as l0sb,
            ):
                for t in range(T16):
                    vjT = l0sb.tile([16, NB * 128], F32, tag="vjT")
                    nc.gpsimd.ap_gather(
                        vjT, VT16, idxw[:, t, :],
                        channels=16, num_elems=N, d=1, num_idxs=NB * 128,
                    )
                    dispT = l0sb.tile([3, NB, 128], F16, tag="dispT")
                    nc.vector.tensor_sub(
                        dispT,
                        vjT[0:3, :].rearrange("d (n i) -> d n i", n=NB),
                        VT16[0:3, t * 128:(t + 1) * 128]
                        .rearrange("d (a i) -> d a i", a=1)
                        .to_broadcast([3, NB, 128]),
                    )
                    nc.sync.dma_start(
                        _ap(TdT, [[2560, 3], [128, NB], [1, 128]],
                            off=t * 3 * NB * 128),
                        dispT,
                    )

                    # theta0 = relu(disp @ w0_disp): 20 matmuls, groups of 4 in PSUM
                    th0 = l0sb.tile([128, NB, SK0], F16, tag="th0")
                    for g0 in range(0, NB, 4):
                        th_ps = l0ps.tile([128, 4, 256], F32, tag="l0ps")
                        for j in range(4):
                            nn = g0 + j
                            nc.tensor.matmul(
                                th_ps[:, j, 0:SK0],
                                dispT[:, nn, :],
                                w01_sb[:, 0:SK0],
                                start=True, stop=True,
                            )
                        nc.scalar.activation(
                            th0[:, g0:g0 + 4, :], th_ps[:, :, 0:SK0], ACTF.Relu
                        )
                    # max over neighbors (pairwise tree, f16)
                    t10 = l0sb.tile([128, 10, SK0], F16, tag="t10")
                    nc.vector.tensor_tensor(
                        out=t10, in0=th0[:, 0:10, :], in1=th0[:, 10:20, :], op=ALU.max
                    )
                    nc.vector.tensor_tensor(
                        out=t10[:, 0:5, :], in0=t10[:, 0:5, :], in1=t10[:, 5:10, :],
                        op=ALU.max,
                    )
                    nc.vector.tensor_tensor(
                        out=t10[:, 0:2, :], in0=t10[:, 0:2, :], in1=t10[:, 2:4, :],
                        op=ALU.max,
                    )
                    nc.vector.tensor_tensor(
                        out=t10[:, 0:1, :], in0=t10[:, 0:1, :], in1=t10[:, 1:2, :],
                        op=ALU.max,
                    )
                    thm = l0sb.tile([128, K0, SUP], F32, tag="thm")  # (k, s) layout
                    nc.vector.tensor_tensor(
                        out=thm.rearrange("p k s -> p s k"),
                        in0=t10[:, 0, :].rearrange("p (s k) -> p s k", s=SUP),
                        in1=t10[:, 4, :].rearrange("p (s k) -> p s k", s=SUP),
                        op=ALU.max,
                    )
                    # fm0 = relu(sum_s thm * w0w)
                    nc.vector.tensor_mul(
                        thm.rearrange("p k s -> p s k"),
                        thm.rearrange("p k s -> p s k"),
                        w0w_sb.rearrange("p (s k) -> p s k", s=SUP),
                    )
                    fm0 = l0sb.tile([128, K0], F32, tag="fm0")
                    nc.vector.tensor_reduce(fm0, thm, axis=AXX, op=ALU.add)
                    nc.vector.tensor_scalar_max(fm0, fm0, 0.0)
                    f0T_ps = l0ps.tile([K0, 128], F32, tag="l0ps")
                    nc.tensor.transpose(f0T_ps, fm0, ident_sb)
                    nc.scalar.activation(fm01T[0:K0, t, :], f0T_ps, ACTF.Copy)

                    # feat1 = [fm0|1] @ [w1;b1]
                    f1_ps = l0ps.tile([128, 512], F32, tag="l0ps")
                    nc.tensor.matmul(
                        f1_ps, fm01T[:, t, :], wb1_sb, start=True, stop=True
                    )
                    nc.scalar.activation(f1c[:, t, :], f1_ps[:, 0:C1], ACTF.Copy)
                    f1s = l0sb.tile([128, 512], F16, tag="f1s")
                    nc.scalar.activation(f1s[:, 0:SC1], f1_ps[:, C1:512], ACTF.Copy)
                    nc.vector.memset(f1s[:, SC1:512], 0.0)
                    nc.sync.dma_start(
                        _ap(T1, [[512, 128], [1, 512]], off=t * 128 * 512), f1s
                    )

            # ---- Phase 3: conv1 (layer 1), per tile ----
            with (
                tc.tile_pool(name="l1_psum", bufs=2, space="PSUM") as l1ps,
                tc.tile_pool(name="l1_sb", bufs=2) as l1sb,
            ):
                for t in range(T16):
                    fg = l1sb.tile([128, NB, 512], F16, tag="fg")
                    for n in range(NB):
                        nc.gpsimd.indirect_dma_start(
                            out=fg[:, n, :], out_offset=None, in_=T1[:, :],
                            in_offset=IOA(ap=NI32[:, t, n:n + 1], axis=0),
                        )
                    dTs = l1sb.tile([3, NB, 128], F16, tag="dTs")
                    nc.sync.dma_start(
                        dTs,
                        _ap(TdT, [[2560, 3], [128, NB], [1, 128]],
                            off=t * 3 * NB * 128),
                    )
                    th1 = l1sb.tile([128, NB, SC1], F16, tag="th1", bufs=2)
                    for g0 in range(0, NB, 4):
                        th_ps = l1ps.tile([128, 4, 512], F32, tag="th1_ps")
                        for j in range(4):
                            nn = g0 + j
                            nc.tensor.matmul(
                                th_ps[:, j, 0:SC1],
                                dTs[:, nn, :],
                                w01_sb[:, SK0:W01],
                                start=True, stop=True,
                            )
                        nc.scalar.activation(
                            th1[:, g0:g0 + 4, :], th_ps[:, :, 0:SC1], ACTF.Relu
                        )
                    # prod = theta1 * feat_support(gathered); tree max over n
                    nc.vector.tensor_mul(th1, th1, fg[:, :, 0:SC1])
                    t10 = l1sb.tile([128, 10, SC1], F16, tag="l1t10", bufs=1)
                    nc.vector.tensor_tensor(
                        out=t10, in0=th1[:, 0:10, :], in1=th1[:, 10:20, :], op=ALU.max
                    )
                    nc.vector.tensor_tensor(
                        out=t10[:, 0:5, :], in0=t10[:, 0:5, :], in1=t10[:, 5:10, :],
                        op=ALU.max,
                    )
                    nc.vector.tensor_tensor(
                        out=t10[:, 0:2, :], in0=t10[:, 0:2, :], in1=t10[:, 2:4, :],
                        op=ALU.max,
                    )
                    nc.vector.tensor_tensor(
                        out=t10[:, 0:1, :], in0=t10[:, 0:1, :], in1=t10[:, 1:2, :],
                        op=ALU.max,
                    )
                    a1t = l1sb.tile([128, C1, SUP], F32, tag="a1t", bufs=1)
                    nc.vector.tensor_tensor(
                        out=a1t.rearrange("p c s -> p s c"),
                        in0=t10[:, 0, :].rearrange("p (s c) -> p s c", s=SUP),
                        in1=t10[:, 4, :].rearrange("p (s c) -> p s c", s=SUP),
                        op=ALU.max,
                    )
                    fm1 = l1sb.tile([128, C1], F32, tag="fm1", bufs=1)
                    nc.vector.tensor_reduce(fm1, a1t, axis=AXX, op=ALU.add)
                    nc.vector.tensor_add(fm1, fm1, f1c[:, t, :])
                    nc.vector.tensor_scalar_max(fm1, fm1, 0.0)
                    fm1h = l1sb.tile([128, 128], F16, tag="fm1h")
                    nc.vector.tensor_copy(fm1h[:, 0:C1], fm1)
                    nc.vector.memset(fm1h[:, C1:128], 0.0)
                    nc.sync.dma_start(
                        _ap(T2, [[128, 128], [1, 128]], off=t * 128 * 128), fm1h
                    )

            ctx_a.close()

            # ---- Phase 4: pooling 1 (sampled 256 vertices) ----
            nc.vector.memset(f2T[C1:C1 + 1, :, :], 1.0)
            with (
                tc.tile_pool(name="p1_sb", bufs=1) as p1sb,
                tc.tile_pool(name="p1_ps", bufs=2, space="PSUM") as p1ps,
            ):
                NIs = p1sb.tile([128, 2, 128], I16)
                NIs32 = p1sb.tile([128, 2, 8], I32)
                for g in range(2):
                    nc.gpsimd.indirect_dma_start(
                        out=NIs[:, g, :], out_offset=None, in_=TNI[:, :],
                        in_offset=IOA(ap=s1pp_sb[:, g:g + 1], axis=0),
                    )
                    nc.gpsimd.indirect_dma_start(
                        out=v1g[:, g, :], out_offset=None, in_=T0[:, :],
                        in_offset=IOA(ap=s1pp_sb[:, g:g + 1], axis=0),
                    )
                nc.vector.tensor_copy(NIs32, NIs[:, :, 0:8])

                fmg = p1sb.tile([128, 2, 8, 128], F16)
                for g in range(2):
                    for n in range(8):
                        nc.gpsimd.indirect_dma_start(
                            out=fmg[:, g, n, :], out_offset=None, in_=T2[:, :],
                            in_offset=IOA(ap=NIs32[:, g, n:n + 1], axis=0),
                        )
                nc.vector.tensor_tensor(
                    out=fmg[:, :, 0:4], in0=fmg[:, :, 0:4], in1=fmg[:, :, 4:8],
                    op=ALU.max,
                )
                nc.vector.tensor_tensor(
                    out=fmg[:, :, 0:2], in0=fmg[:, :, 0:2], in1=fmg[:, :, 2:4],
                    op=ALU.max,
                )
                nc.vector.tensor_tensor(
                    out=fmg[:, :, 0:1], in0=fmg[:, :, 0:1], in1=fmg[:, :, 1:2],
                    op=ALU.max,
                )
                fm1p = p1sb.tile([128, 2, C1], F32)
                nc.vector.tensor_copy(fm1p, fmg[:, :, 0, 0:C1])

                nc.sync.dma_start(_ap(T0v1, [[64, 128], [8192, 2], [1, 64]]), v1g)
                for g in range(2):
                    fT_ps = p1ps.tile([C1, 128], F32, tag="fT_ps")
                    nc.tensor.transpose(fT_ps, fm1p[:, g, :], ident_sb)
                    nc.scalar.activation(f2T[0:C1, g, :], fT_ps, ACTF.Copy)

            # ---- Phase 5: KNN on 256 pooled vertices ----
            with (
                tc.tile_pool(name="k2_ps", bufs=2, space="PSUM") as k2ps,
                tc.tile_pool(name="k2_ps1", bufs=1, space="PSUM") as k2ps1,
                tc.tile_pool(name="k2_sb", bufs=2) as k2sb,
                tc.tile_pool(name="k2_sb1", bufs=1) as k2sb1,
            ):
                lhsT1 = k2sb1.tile([4, N1], F32)
                rhs1 = k2sb1.tile([4, N1], F32)
                vT_ps = k2ps1.tile([3, N1], F32)
                for g in range(2):
                    nc.tensor.transpose(
                        vT_ps[:, g * 128:(g + 1) * 128], v1g[:, g, 0:3], ident_sb
                    )
                nc.scalar.activation(lhsT1[0:3, :], vT_ps, ACTF.Copy)
                nc.sync.dma_start(lhsT1[3:4, :], ones_row[:, 0:N1])
                nc.vector.tensor_scalar_mul(rhs1[0:3, :], lhsT1[0:3, :], 2.0)
                v1sq = k2sb1.tile([128, 2, 3], F32)
                nc.vector.tensor_mul(v1sq, v1g[:, :, 0:3], v1g[:, :, 0:3])
                q1 = k2sb1.tile([128, 2], F32)
                nc.vector.tensor_reduce(q1, v1sq, axis=AXX, op=ALU.add)
                q1T_ps = k2ps1.tile([2, 128], F32)
                nc.tensor.transpose(q1T_ps, q1, ident_sb)
                q1T = k2sb1.tile([2, 128], F32)
                nc.scalar.activation(q1T, q1T_ps, ACTF.Copy, scale=-1.0)
                nc.sync.dma_start(
                    rhs1[3:4, :].rearrange("a (g p) -> a g p", p=128), q1T
                )
                for g in range(2):
                    m2_ps = k2ps.tile([128, N1], F32, tag="m2_ps")
                    nc.tensor.matmul(
                        m2_ps, lhsT1[:, g * 128:(g + 1) * 128], rhs1,
                        start=True, stop=True,
                    )
                    m2 = k2sb.tile([128, N1], F32, tag="m2")
                    nc.scalar.activation(m2, m2_ps, ACTF.Copy)
                    nc.vector.tensor_add(
                        m2[:, g * 128:(g + 1) * 128],
                        m2[:, g * 128:(g + 1) * 128], neg_i_sb,
                    )
                    for r in range(3):
                        rv2 = k2sb.tile([128, 8], F32, tag="rv2")
                        nc.vector.max(out=rv2, in_=m2)
                        nc.vector.max_index(
                            out=NI2[:, g, r * 8:(r + 1) * 8], in_max=rv2, in_values=m2
                        )
                        if r < 2:
                            nc.vector.match_replace(
                                out=m2, in_to_replace=rv2, in_values=m2, imm_value=NEG
                            )

                nc.vector.tensor_copy(NI232, NI2[:, :, 0:NB])
                nc.sync.dma_start(
                    _ap(TNI2, [[128, 128], [16384, 2], [1, 24]]),
                    NI2[:, :, :].bitcast(I16),
                )
                for g in range(2):
                    nc.sync.dma_start(
                        _ap(FC, [[1, 128], [128, NB], [1, 1]], off=g * 2560),
                        NI2[:, g:g + 1, 0:NB].bitcast(I16).rearrange("p a n -> p n a"),
                    )
                    nc.sync.dma_start(
                        _ap(FCT, [[160, 16], [1, 160], [1, 1]], off=g * 2560),
                        _ap(FC, [[1, 16], [16, 160], [1, 1]], off=g * 2560),
                    )
                nc.sync.dma_start(idxw2, _ap(FCT, [[160, 16], [2560, 2], [1, 160]]))

            # ---- Phase 6: feat2 + conv2 (layer 2) ----
            with (
                tc.tile_pool(name="l2_ps", bufs=2, space="PSUM") as l2ps,
                tc.tile_pool(name="l2_sb", bufs=1) as l2sb,
                tc.tile_pool(name="l2_sb2", bufs=2) as l2sb2,
            ):
                f2c = l2sb.tile([128, 2, C2], F32)
                for g in range(2):
                    f2_ps = l2ps.tile([128, 4, 512], F32, tag="big_ps", bufs=2)
                    for j in range(4):
                        nc.tensor.matmul(
                            f2_ps[:, j, :], f2T[:, g, :],
                            wb2_sb[:, j * 512:(j + 1) * 512],
                            start=True, stop=True,
                        )
                    f2flat = f2_ps.rearrange("p a b -> p (a b)")
                    nc.scalar.activation(f2c[:, g, :], f2flat[:, 0:C2], ACTF.Copy)
                    f2s = l2sb2.tile([128, SC2], F16, tag="f2s")
                    nc.scalar.activation(f2s, f2flat[:, C2:2048], ACTF.Copy)
                    nc.sync.dma_start(
                        _ap(T3, [[SC2, 128], [1, SC2]], off=g * 128 * SC2), f2s
                    )

                VT116 = l2sb.tile([16, N1], F32)
                nc.vector.memset(VT116, 0.0)
                nc.sync.dma_start(VT116[0:3, :], _ap(T0v1, [[1, 3], [64, N1]]))
                for g in range(2):
                    vjT2 = l2sb2.tile([16, NB * 128], F32, tag="vjT2", bufs=1)
                    nc.gpsimd.ap_gather(
                        vjT2, VT116, idxw2[:, g, :],
                        channels=16, num_elems=N1, d=1, num_idxs=NB * 128,
                    )
                    dispT2 = l2sb2.tile([3, NB, 128], F16, tag="dispT2", bufs=1)
                    nc.vector.tensor_sub(
                        dispT2,
                        vjT2[0:3, :].rearrange("d (n i) -> d n i", n=NB),
                        VT116[0:3, g * 128:(g + 1) * 128]
                        .rearrange("d (a i) -> d a i", a=1)
                        .to_broadcast([3, NB, 128]),
                    )

                    acc = l2sb2.tile([128, SC2], F16, tag="acc", bufs=1)
                    th2 = None
                    for h in range(2):
                        fg2 = l2sb2.tile([128, 10, SC2], F16, tag="fg2", bufs=2)
                        for j in range(10):
                            nn = h * 10 + j
                            nc.gpsimd.indirect_dma_start(
                                out=fg2[:, j, :], out_offset=None, in_=T3[:, :],
                                in_offset=IOA(ap=NI232[:, g, nn:nn + 1], axis=0),
                            )
                        th2 = l2sb2.tile([128, 10, SC2], F16, tag="th2", bufs=1)
                        for j in range(10):
                            nn = h * 10 + j
                            th2_ps = l2ps.tile([128, 4, 512], F32, tag="big_ps", bufs=2)
                            for u in range(4):
                                w = min(512, SC2 - u * 512)
                                nc.tensor.matmul(
                                    th2_ps[:, u, 0:w],
                                    dispT2[:, nn, :],
                                    w2d_sb[:, u * 512:u * 512 + w],
                                    start=True, stop=True,
                                )
                            nc.scalar.activation(
                                th2[:, j, :],
                                th2_ps.rearrange("p a b -> p (a b)")[:, 0:SC2],
                                ACTF.Relu,
                            )
                        nc.vector.tensor_mul(th2, th2, fg2)
                        nc.vector.tensor_tensor(
                            out=th2[:, 0:5, :], in0=th2[:, 0:5, :], in1=th2[:, 5:10, :],
                            op=ALU.max,
                        )
                        nc.vector.tensor_tensor(
                            out=th2[:, 0:2, :], in0=th2[:, 0:2, :], in1=th2[:, 2:4, :],
                            op=ALU.max,
                        )
                        nc.vector.tensor_tensor(
                            out=th2[:, 0:1, :], in0=th2[:, 0:1, :], in1=th2[:, 1:2, :],
                            op=ALU.max,
                        )
                        if h == 0:
                            nc.vector.tensor_tensor(
                                out=acc, in0=th2[:, 0, :], in1=th2[:, 4, :], op=ALU.max
                            )
                        else:
                            nc.vector.tensor_tensor(
                                out=th2[:, 0:1, :], in0=th2[:, 0:1, :],
                                in1=th2[:, 4:5, :], op=ALU.max,
                            )
                    a2t = l2sb2.tile([128, C2, SUP], F32, tag="a2t", bufs=1)
                    nc.vector.tensor_tensor(
                        out=a2t.rearrange("p c s -> p s c"),
                        in0=acc.rearrange("p (s c) -> p s c", s=SUP),
                        in1=th2[:, 0, :].rearrange("p (s c) -> p s c", s=SUP),
                        op=ALU.max,
                    )
                    fm2 = l2sb2.tile([128, C2], F32, tag="fm2", bufs=1)
                    nc.vector.tensor_reduce(fm2, a2t, axis=AXX, op=ALU.add)
                    nc.vector.tensor_add(fm2, fm2, f2c[:, g, :])
                    nc.vector.tensor_scalar_max(fm2, fm2, 0.0)
                    fm2h = l2sb2.tile([128, C2], F16, tag="fm2h", bufs=1)
                    nc.vector.tensor_copy(fm2h, fm2)
                    nc.sync.dma_start(
                        _ap(T4, [[C2, 128], [1, C2]], off=g * 128 * C2), fm2h
                    )

            # ---- Phase 7: pooling 2 + outputs ----
            with tc.tile_pool(name="p2_sb", bufs=1) as p2sb:
                NIs2 = p2sb.tile([16, 128], I16)
                nc.gpsimd.indirect_dma_start(
                    out=NIs2, out_offset=None, in_=TNI2[:, :],
                    in_offset=IOA(ap=s2pp_sb[:, 0:1], axis=0),
                )
                NIs232 = p2sb.tile([16, 16], I32)
                nc.vector.tensor_copy(NIs232, NIs2[:, 0:16])
                fmg2 = p2sb.tile([16, 16, C2], F16)
                for n in range(16):
                    nc.gpsimd.indirect_dma_start(
                        out=fmg2[:, n, :], out_offset=None, in_=T4[:, :],
                        in_offset=IOA(ap=NIs232[:, n:n + 1], axis=0),
                    )
                nc.vector.tensor_tensor(
                    out=fmg2[:, 0:8, :], in0=fmg2[:, 0:8, :], in1=fmg2[:, 8:16, :],
                    op=ALU.max,
                )
                nc.vector.tensor_tensor(
                    out=fmg2[:, 0:4, :], in0=fmg2[:, 0:4, :], in1=fmg2[:, 4:8, :],
                    op=ALU.max,
                )
                nc.vector.tensor_tensor(
                    out=fmg2[:, 0:2, :], in0=fmg2[:, 0:2, :], in1=fmg2[:, 2:4, :],
                    op=ALU.max,
                )
                nc.vector.tensor_tensor(
                    out=fmg2[:, 0:1, :], in0=fmg2[:, 0:1, :], in1=fmg2[:, 1:2, :],
                    op=ALU.max,
                )
                fm3f = p2sb.tile([16, C2], F32)
                nc.vector.tensor_copy(fm3f, fmg2[:, 0, :])
                nc.sync.dma_start(out_fm3[:, :], fm3f[0:N2, :])

                v2g = p2sb.tile([16, 64], F32)
                nc.gpsimd.indirect_dma_start(
                    out=v2g, out_offset=None, in_=T0v1[:, :],
                    in_offset=IOA(ap=s2pp_sb[:, 0:1], axis=0),
                )
                nc.sync.dma_start(out_v2[:, :], v2g[0:N2, 0:3])

    nc.compile()
    return nc


_NC_CACHE = None


def _get_nc():
    global _NC_CACHE
    if _NC_CACHE is None:
        _NC_CACHE = build_nc()
    return _NC_CACHE


def make_in_maps(inputs):
    v = np.asarray(inputs["vertices"], dtype=np.float32)
    w0_weights = np.asarray(inputs["w0_weights"], dtype=np.float32)
    w0_disp = np.asarray(inputs["w0_disp"], dtype=np.float32)
    w1_weights = np.asarray(inputs["w1_weights"], dtype=np.float32)
    w1_bias = np.asarray(inputs["w1_bias"], dtype=np.float32)
    w1_disp = np.asarray(inputs["w1_disp"], dtype=np.float32)
    w2_weights = np.asarray(inputs["w2_weights"], dtype=np.float32)
    w2_bias = np.asarray(inputs["w2_bias"], dtype=np.float32)
    w2_disp = np.asarray(inputs["w2_disp"], dtype=np.float32)

    shared = {
        "w01_disp": np.ascontiguousarray(
            np.concatenate([w0_disp, w1_disp], axis=1)
        ).astype(np.float16),
        "w2_disp": np.ascontiguousarray(w2_disp).astype(np.float16),
        "w0w": np.ascontiguousarray(
            np.tile(w0_weights.reshape(1, SUP * K0), (128, 1))
        ),
        "wb1": np.ascontiguousarray(
            np.concatenate([w1_weights, w1_bias[None, :]], axis=0)
        ),
        "wb2": np.ascontiguousarray(
            np.concatenate([w2_weights, w2_bias[None, :]], axis=0)
        ),
        "neg_i": (np.eye(128, dtype=np.float32) * NEG).astype(np.float32),
        "identity": np.eye(128, dtype=np.float32),
        "ones_row": np.ones((1, N), dtype=np.float32),
        "samp1_pp": np.ascontiguousarray(
            SAMPLE1.reshape(2, 128).T.astype(np.int32)
        ),
        "samp2_pp": SAMPLE2.reshape(16, 1).astype(np.int32),
    }
    in_maps = []
    for b in range(v.shape[0]):
        m = dict(shared)
        m["vertices"] = np.ascontiguousarray(v[b])
        in_maps.append(m)
    return in_maps


def kernel(**inputs):
    from concourse import bass_utils

    nc = _get_nc()
    in_maps = make_in_maps(inputs)
    res = bass_utils.run_bass_kernel_spmd(nc, in_maps, core_ids=list(range(BS)))
    v2 = np.stack([res.results[b]["out_v2"] for b in range(BS)])
    fm3 = np.stack([res.results[b]["out_fm3"] for b in range(BS)])
    return v2.astype(np.float32), fm3.astype(np.float32)


# revision 17
# speedup vs baseline: 1.0089x; 1.0089x over previous
"""Trainium2 Bass kernel for the GNN point-cloud encoder (KNN graph conv x3 + 2 poolings).

Data parallel over batch: 8 batch elements -> 8 NeuronCores, one full encoder per core.
Accepts FULL inputs, returns FULL outputs (v2, fm3) as numpy arrays.
"""

import contextlib
import sys

sys.path.insert(0, "/opt/trn_rl_repo")

import numpy as np

import concourse.bass as bass
import concourse.mybir as mybir
import concourse.tile as tile
from concourse import bacc

F32 = mybir.dt.float32
F16 = mybir.dt.float16
U16 = mybir.dt.uint16
I16 = mybir.dt.int16
I32 = mybir.dt.int32
ALU = mybir.AluOpType
ACTF = mybir.ActivationFunctionType
AXX = mybir.AxisListType.X
IOA = bass.IndirectOffsetOnAxis

# Problem constants (hardcoded per the harness contract)
BS = 8
N = 2048
NB = 20          # neighbor count for conv layers
SUP = 7          # support_num
K0, C1, C2 = 32, 64, 256
N1 = 256         # vertices after pooling 1 (2048 / 8)
N2 = 16          # vertices after pooling 2 (256 / 16)
T16 = N // 128   # 16 row tiles
NEG = -3.0e38

SK0 = SUP * K0            # 224
SC1 = SUP * C1            # 448
SC2 = SUP * C2            # 1792
W01 = SK0 + SC1           # 672

SAMPLE1 = np.random.default_rng(1).permutation(N)[:N1]          # pooling 1 sample
SAMPLE2 = np.random.default_rng(2).permutation(N1)[:N2]         # pooling 2 sample


def _ap(t, dims, off=0):
    """Raw access pattern on the tensor behind AP `t` (DRAM only; steps in elements)."""
    return bass.AP(tensor=t.tensor, offset=t.offset + off, ap=[list(d) for d in dims])


def build_nc():
    nc = bacc.Bacc("TRN2", target_bir_lowering=False, debug=False)

    verts = nc.dram_tensor("vertices", [N, 3], F32, kind="ExternalInput")
    w01_disp = nc.dram_tensor("w01_disp", [3, W01], F16, kind="ExternalInput")
    w2_disp = nc.dram_tensor("w2_disp", [3, SC2], F16, kind="ExternalInput")
    w0w = nc.dram_tensor("w0w", [128, SK0], F32, kind="ExternalInput")
    wb1 = nc.dram_tensor("wb1", [K0 + 1, (SUP + 1) * C1], F32, kind="ExternalInput")
    wb2 = nc.dram_tensor("wb2", [C1 + 1, (SUP + 1) * C2], F32, kind="ExternalInput")
    neg_i = nc.dram_tensor("neg_i", [128, 128], F32, kind="ExternalInput")
    identity = nc.dram_tensor("identity", [128, 128], F32, kind="ExternalInput")
    samp1_pp = nc.dram_tensor("samp1_pp", [128, 2], I32, kind="ExternalInput")
    samp2_pp = nc.dram_tensor("samp2_pp", [16, 1], I32, kind="ExternalInput")
    ones_row = nc.dram_tensor("ones_row", [1, N], F32, kind="ExternalInput")

    out_v2 = nc.dram_tensor("out_v2", [N2, 3], F32, kind="ExternalOutput")
    out_fm3 = nc.dram_tensor("out_fm3", [N2, C2], F32, kind="ExternalOutput")

    with tile.TileContext(nc) as tc:
        with contextlib.ExitStack() as ctx:
            dram = ctx.enter_context(tc.tile_pool(name="dram", bufs=1, space="DRAM"))
            const = ctx.enter_context(tc.tile_pool(name="const", bufs=1))
            pers = ctx.enter_context(tc.tile_pool(name="pers", bufs=1))
            ctx_a = ctx.enter_context(contextlib.ExitStack())

            # ---- DRAM scratch tables ----
            T0 = dram.tile([N, 64], F32, name="T0", uniquify=False)          # padded coords
            T1 = dram.tile([N, 512], F16, name="T1", uniquify=False)         # feat1 support
            T2 = dram.tile([N, 128], F16, name="T2", uniquify=False)         # fm1 padded
            T3 = dram.tile([N1, SC2], F16, name="T3", uniquify=False)        # feat2 support
            T4 = dram.tile([N1, C2], F16, name="T4", uniquify=False)         # fm2
            TNI = dram.tile([N, 128], I16, name="TNI", uniquify=False)       # NI padded rows
            TNI2 = dram.tile([N1, 128], I16, name="TNI2", uniquify=False)
            T0v1 = dram.tile([N1, 64], F32, name="T0v1", uniquify=False)     # pooled coords
            TdT = dram.tile([T16, 3, NB, 128], F16, name="TdT", uniquify=False)
            FB = dram.tile([T16 * 2560], I16, name="FB", uniquify=False)     # q-major lists
            FBT = dram.tile([T16 * 2560], I16, name="FBT", uniquify=False)   # wrapped
            FC = dram.tile([2 * 2560], I16, name="FC", uniquify=False)       # knn2 lists
            FCT = dram.tile([2 * 2560], I16, name="FCT", uniquify=False)

            # ---- constants / weights to SBUF ----
            neg_i_sb = const.tile([128, 128], F32)
            nc.sync.dma_start(neg_i_sb, neg_i[:, :])
            ident_sb = const.tile([128, 128], F32)
            nc.sync.dma_start(ident_sb, identity[:, :])
            w01_sb = const.tile([3, W01], F16)
            nc.sync.dma_start(w01_sb, w01_disp[:, :])
            w2d_sb = const.tile([3, SC2], F16)
            nc.sync.dma_start(w2d_sb, w2_disp[:, :])
            w0w_sb = const.tile([128, SK0], F32)
            nc.sync.dma_start(w0w_sb, w0w[:, :])
            wb1_sb = const.tile([K0 + 1, (SUP + 1) * C1], F32)
            nc.sync.dma_start(wb1_sb, wb1[:, :])
            wb2_sb = const.tile([C1 + 1, (SUP + 1) * C2], F32)
            nc.sync.dma_start(wb2_sb, wb2[:, :])
            s1pp_sb = const.tile([128, 2], I32)
            nc.sync.dma_start(s1pp_sb, samp1_pp[:, :])
            s2pp_sb = const.tile([16, 1], I32)
            nc.sync.dma_start(s2pp_sb, samp2_pp[:, :])

            # per-layer state: pers_a dies after conv1, pers stays to the end
            pers_a = ctx_a.enter_context(tc.tile_pool(name="pers_a", bufs=1))
            NI = pers_a.tile([128, T16, 24], U16)
            NI32 = pers_a.tile([128, T16, NB], I32)
            V_sb = pers_a.tile([128, T16, 3], F32)     # vertex-major
            nc.sync.dma_start(V_sb, _ap(verts[:, :], [[3, 128], [384, T16], [1, 3]]))
            VT16 = pers_a.tile([16, N], F32)           # rows 0:3 = V.T, rest 0
            nc.vector.memset(VT16, 0.0)
            nc.sync.dma_start(VT16[0:3, :], _ap(verts[:, :], [[1, 3], [3, N]]))
            idxw = pers_a.tile([16, T16, 160], I16)    # wrap-16 q-lists per tile
            f1c = pers_a.tile([128, T16, C1], F32)
            fm01T = pers_a.tile([K0 + 1, T16, 128], F32)
            f2T = pers.tile([C1 + 1, 2, 128], F32)
            v1g = pers.tile([128, 2, 64], F32)
            NI2 = pers.tile([128, 2, 24], U16)
            NI232 = pers.tile([128, 2, NB], I32)
            idxw2 = pers.tile([16, 2, 160], I16)

            # ---- Phase 0 + 1: prep and KNN over 2048 vertices ----
            with (
                tc.tile_pool(name="knn_psum", bufs=2, space="PSUM") as kpsum,
                tc.tile_pool(name="knn_sb", bufs=2) as ksb,
                tc.tile_pool(name="prep_sb", bufs=1) as psb,
            ):
                lhsT4 = psb.tile([4, N], F32)
                nc.sync.dma_start(lhsT4[0:3, :], _ap(verts[:, :], [[1, 3], [3, N]]))
                nc.sync.dma_start(lhsT4[3:4, :], ones_row[:, :])
                rhs4 = psb.tile([4, N], F32)
                nc.vector.tensor_scalar_mul(rhs4[0:3, :], lhsT4[0:3, :], 2.0)

                # q_j = |v_j|^2 -> rhs4 row 3 as -q
                vsq = psb.tile([128, T16, 3], F32)
                nc.vector.tensor_mul(vsq, V_sb, V_sb)
                q_vm = psb.tile([128, T16], F32)
                nc.vector.tensor_reduce(q_vm, vsq, axis=AXX, op=ALU.add)
                qT_ps = kpsum.tile([T16, 128], F32, tag="m_ps")
                nc.tensor.transpose(qT_ps, q_vm, ident_sb)
                qT_sb = psb.tile([T16, 128], F32)
                nc.scalar.activation(qT_sb, qT_ps, ACTF.Copy, scale=-1.0)
                nc.sync.dma_start(
                    rhs4[3:4, :].rearrange("a (t p) -> a t p", p=128), qT_sb
                )

                # zero T0 padding then write coords
                zpad = psb.tile([128, 1024], F32)
                nc.vector.memset(zpad, 0.0)
                nc.sync.dma_start(_ap(T0, [[1024, 128], [1, 1024]]), zpad)
                nc.sync.dma_start(_ap(T0, [[64, 128], [8192, T16], [1, 3]]), V_sb)

                for t in range(T16):
                    m_ps = kpsum.tile([128, 4, 512], F32, tag="m_ps")
                    for j in range(4):
                        nc.tensor.matmul(
                            m_ps[:, j, :],
                            lhsT4[:, t * 128:(t + 1) * 128],
                            rhs4[:, j * 512:(j + 1) * 512],
                            start=True, stop=True,
                        )
                    m_sb = ksb.tile([128, N], F32, tag="m_sb")
                    nc.scalar.activation(
                        m_sb, m_ps.rearrange("p a b -> p (a b)"), ACTF.Copy
                    )
                    nc.vector.tensor_add(
                        m_sb[:, t * 128:(t + 1) * 128],
                        m_sb[:, t * 128:(t + 1) * 128],
                        neg_i_sb,
                    )
                    for r in range(3):
                        rv = ksb.tile([128, 8], F32, tag="rv")
                        nc.vector.max(out=rv, in_=m_sb)
                        nc.vector.max_index(
                            out=NI[:, t, r * 8:(r + 1) * 8], in_max=rv, in_values=m_sb
                        )
                        if r < 2:
                            nc.vector.match_replace(
                                out=m_sb, in_to_replace=rv, in_values=m_sb,
                                imm_value=NEG,
                            )

            nc.vector.tensor_copy(NI32, NI[:, :, 0:NB])
            # NI -> TNI (padded rows, for pooling-1 sampled lookup)
            nc.sync.dma_start(
                _ap(TNI, [[128, 128], [16384, T16], [1, 24]]),
                NI[:, :, :].bitcast(I16),
            )
            # wrap-16 q-ordered index lists for ap_gather (via elementwise DMAs)
            for t in range(T16):
                nc.sync.dma_start(
                    _ap(FB, [[1, 128], [128, NB], [1, 1]], off=t * 2560),
                    NI[:, t:t + 1, 0:NB].bitcast(I16).rearrange("p a n -> p n a"),
                )
                nc.sync.dma_start(
                    _ap(FBT, [[160, 16], [1, 160], [1, 1]], off=t * 2560),
                    _ap(FB, [[1, 16], [16, 160], [1, 1]], off=t * 2560),
                )
                nc.sync.dma_start(
                    idxw[:, t, :], _ap(FBT, [[160, 16], [1, 160]], off=t * 2560)
                )

            # ---- Phase 2: conv0 (layer 0) + feat1, per tile ----
            nc.vector.memset(fm01T[K0:K0 + 1, :, :], 1.0)
            with (
                tc.tile_pool(name="l0_psum", bufs=3, space="PSUM") as l0ps,
                tc.tile_pool(name="l0_sb", bufs=2) as l0sb,
            ):
                for t in range(T16):
                    vjT = l0sb.tile([16, NB * 128], F32, tag="vjT")
                    nc.gpsimd.ap_gather(
                        vjT, VT16, idxw[:, t, :],
                        channels=16, num_elems=N, d=1, num_idxs=NB * 128,
                    )
                    dispT = l0sb.tile([3, NB, 128], F16, tag="dispT")
                    nc.vector.tensor_sub(
                        dispT,
                        vjT[0:3, :].rearrange("d (n i) -> d n i", n=NB),
                        VT16[0:3, t * 128:(t + 1) * 128]
                        .rearrange("d (a i) -> d a i", a=1)
                        .to_broadcast([3, NB, 128]),
                    )
                    nc.sync.dma_start(
                        _ap(TdT, [[2560, 3], [128, NB], [1, 128]],
                            off=t * 3 * NB * 128),
                        dispT,
                    )

                    # theta0 = relu(disp @ w0_disp): 20 matmuls, groups of 4 in PSUM
                    th0 = l0sb.tile([128, NB, SK0], F16, tag="th0")
                    for g0 in range(0, NB, 4):
                        th_ps = l0ps.tile([128, 4, 256], F32, tag="l0ps")
                        for j in range(4):
                            nn = g0 + j
                            nc.tensor.matmul(
                                th_ps[:, j, 0:SK0],
                                dispT[:, nn, :],
                                w01_sb[:, 0:SK0],
                                start=True, stop=True,
                            )
                        nc.scalar.activation(
                            th0[:, g0:g0 + 4, :], th_ps[:, :, 0:SK0], ACTF.Relu
                        )
                    # max over neighbors (pairwise tree, f16)
                    t10 = l0sb.tile([128, 10, SK0], F16, tag="t10")
                    nc.vector.tensor_tensor(
                        out=t10, in0=th0[:, 0:10, :], in1=th0[:, 10:20, :], op=ALU.max
                    )
                    nc.vector.tensor_tensor(
                        out=t10[:, 0:5, :], in0=t10[:, 0:5, :], in1=t10[:, 5:10, :],
                        op=ALU.max,
                    )
                    nc.vector.tensor_tensor(
                        out=t10[:, 0:2, :], in0=t10[:, 0:2, :], in1=t10[:, 2:4, :],
                        op=ALU.max,
                    )
                    nc.vector.tensor_tensor(
                        out=t10[:, 0:1, :], in0=t10[:, 0:1, :], in1=t10[:, 1:2, :],
                        op=ALU.max,
                    )
                    thm = l0sb.tile([128, K0, SUP], F32, tag="thm")  # (k, s) layout
                    nc.vector.tensor_tensor(
                        out=thm.rearrange("p k s -> p s k"),
                        in0=t10[:, 0, :].rearrange("p (s k) -> p s k", s=SUP),
                        in1=t10[:, 4, :].rearrange("p (s k) -> p s k", s=SUP),
                        op=ALU.max,
                    )
                    # fm0 = relu(sum_s thm * w0w)
                    nc.vector.tensor_mul(
                        thm.rearrange("p k s -> p s k"),
                        thm.rearrange("p k s -> p s k"),
                        w0w_sb.rearrange("p (s k) -> p s k", s=SUP),
                    )
                    fm0 = l0sb.tile([128, K0], F32, tag="fm0")
                    nc.vector.tensor_reduce(fm0, thm, axis=AXX, op=ALU.add)
                    nc.vector.tensor_scalar_max(fm0, fm0, 0.0)
                    f0T_ps = l0ps.tile([K0, 128], F32, tag="l0ps")
                    nc.tensor.transpose(f0T_ps, fm0, ident_sb)
                    nc.scalar.activation(fm01T[0:K0, t, :], f0T_ps, ACTF.Copy)

                    # feat1 = [fm0|1] @ [w1;b1]
                    f1_ps = l0ps.tile([128, 512], F32, tag="l0ps")
                    nc.tensor.matmul(
                        f1_ps, fm01T[:, t, :], wb1_sb, start=True, stop=True
                    )
                    nc.scalar.activation(f1c[:, t, :], f1_ps[:, 0:C1], ACTF.Copy)
                    f1s = l0sb.tile([128, 512], F16, tag="f1s")
                    nc.scalar.activation(f1s[:, 0:SC1], f1_ps[:, C1:512], ACTF.Copy)
                    nc.vector.memset(f1s[:, SC1:512], 0.0)
                    nc.sync.dma_start(
                        _ap(T1, [[512, 128], [1, 512]], off=t * 128 * 512), f1s
                    )

            # ---- Phase 3: conv1 (layer 1), per tile ----
            with (
                tc.tile_pool(name="l1_psum", bufs=2, space="PSUM") as l1ps,
                tc.tile_pool(name="l1_sb", bufs=2) as l1sb,
            ):
                for t in range(T16):
                    fg = l1sb.tile([128, NB, 512], F16, tag="fg")
                    for n in range(NB):
                        nc.gpsimd.indirect_dma_start(
                            out=fg[:, n, :], out_offset=None, in_=T1[:, :],
                            in_offset=IOA(ap=NI32[:, t, n:n + 1], axis=0),
                        )
                    dTs = l1sb.tile([3, NB, 128], F16, tag="dTs")
                    nc.sync.dma_start(
                        dTs,
                        _ap(TdT, [[2560, 3], [128, NB], [1, 128]],
                            off=t * 3 * NB * 128),
                    )
                    th1 = l1sb.tile([128, NB, SC1], F16, tag="th1", bufs=2)
                    for g0 in range(0, NB, 4):
                        th_ps = l1ps.tile([128, 4, 512], F32, tag="th1_ps")
                        for j in range(4):
                            nn = g0 + j
                            nc.tensor.matmul(
                                th_ps[:, j, 0:SC1],
                                dTs[:, nn, :],
                                w01_sb[:, SK0:W01],
                                start=True, stop=True,
                            )
                        nc.scalar.activation(
                            th1[:, g0:g0 + 4, :], th_ps[:, :, 0:SC1], ACTF.Relu
                        )
                    # prod = theta1 * feat_support(gathered); tree max over n
                    nc.vector.tensor_mul(th1, th1, fg[:, :, 0:SC1])
                    t10 = l1sb.tile([128, 10, SC1], F16, tag="l1t10", bufs=1)
                    nc.vector.tensor_tensor(
                        out=t10, in0=th1[:, 0:10, :], in1=th1[:, 10:20, :], op=ALU.max
                    )
                    nc.vector.tensor_tensor(
                        out=t10[:, 0:5, :], in0=t10[:, 0:5, :], in1=t10[:, 5:10, :],
                        op=ALU.max,
                    )
                    nc.vector.tensor_tensor(
                        out=t10[:, 0:2, :], in0=t10[:, 0:2, :], in1=t10[:, 2:4, :],
                        op=ALU.max,
                    )
                    nc.vector.tensor_tensor(
                        out=t10[:, 0:1, :], in0=t10[:, 0:1, :], in1=t10[:, 1:2, :],
                        op=ALU.max,
                    )
                    a1t = l1sb.tile([128, C1, SUP], F32, tag="a1t", bufs=1)
                    nc.vector.tensor_tensor(
                        out=a1t.rearrange("p c s -> p s c"),
                        in0=t10[:, 0, :].rearrange("p (s c) -> p s c", s=SUP),
                        in1=t10[:, 4, :].rearrange("p (s c) -> p s c", s=SUP),
                        op=ALU.max,
                    )
                    fm1 = l1sb.tile([128, C1], F32, tag="fm1", bufs=1)
                    nc.vector.tensor_reduce(fm1, a1t, axis=AXX, op=ALU.add)
                    nc.vector.tensor_add(fm1, fm1, f1c[:, t, :])
                    nc.vector.tensor_scalar_max(fm1, fm1, 0.0)
                    fm1h = l1sb.tile([128, 128], F16, tag="fm1h")
                    nc.vector.tensor_copy(fm1h[:, 0:C1], fm1)
                    nc.vector.memset(fm1h[:, C1:128], 0.0)
                    nc.sync.dma_start(
                        _ap(T2, [[128, 128], [1, 128]], off=t * 128 * 128), fm1h
                    )

            ctx_a.close()

            # ---- Phase 4: pooling 1 (sampled 256 vertices) ----
            nc.vector.memset(f2T[C1:C1 + 1, :, :], 1.0)
            with (
                tc.tile_pool(name="p1_sb", bufs=1) as p1sb,
                tc.tile_pool(name="p1_ps", bufs=2, space="PSUM") as p1ps,
            ):
                NIs = p1sb.tile([128, 2, 128], I16)
                NIs32 = p1sb.tile([128, 2, 8], I32)
                for g in range(2):
                    nc.gpsimd.indirect_dma_start(
                        out=NIs[:, g, :], out_offset=None, in_=TNI[:, :],
                        in_offset=IOA(ap=s1pp_sb[:, g:g + 1], axis=0),
                    )
                    nc.gpsimd.indirect_dma_start(
                        out=v1g[:, g, :], out_offset=None, in_=T0[:, :],
                        in_offset=IOA(ap=s1pp_sb[:, g:g + 1], axis=0),
                    )
                nc.vector.tensor_copy(NIs32, NIs[:, :, 0:8])

                fmg = p1sb.tile([128, 2, 8, 128], F16)
                for g in range(2):
                    for n in range(8):
                        nc.gpsimd.indirect_dma_start(
                            out=fmg[:, g, n, :], out_offset=None, in_=T2[:, :],
                            in_offset=IOA(ap=NIs32[:, g, n:n + 1], axis=0),
                        )
                nc.vector.tensor_tensor(
                    out=fmg[:, :, 0:4], in0=fmg[:, :, 0:4], in1=fmg[:, :, 4:8],
                    op=ALU.max,
                )
                nc.vector.tensor_tensor(
                    out=fmg[:, :, 0:2], in0=fmg[:, :, 0:2], in1=fmg[:, :, 2:4],
                    op=ALU.max,
                )
                nc.vector.tensor_tensor(
                    out=fmg[:, :, 0:1], in0=fmg[:, :, 0:1], in1=fmg[:, :, 1:2],
                    op=ALU.max,
                )
                fm1p = p1sb.tile([128, 2, C1], F32)
                nc.vector.tensor_copy(fm1p, fmg[:, :, 0, 0:C1])

                nc.sync.dma_start(_ap(T0v1, [[64, 128], [8192, 2], [1, 64]]), v1g)
                for g in range(2):
                    fT_ps = p1ps.tile([C1, 128], F32, tag="fT_ps")
                    nc.tensor.transpose(fT_ps, fm1p[:, g, :], ident_sb)
                    nc.scalar.activation(f2T[0:C1, g, :], fT_ps, ACTF.Copy)

            # ---- Phase 5: KNN on 256 pooled vertices ----
            with (
                tc.tile_pool(name="k2_ps", bufs=2, space="PSUM") as k2ps,
                tc.tile_pool(name="k2_ps1", bufs=1, space="PSUM") as k2ps1,
                tc.tile_pool(name="k2_sb", bufs=2) as k2sb,
                tc.tile_pool(name="k2_sb1", bufs=1) as k2sb1,
            ):
                lhsT1 = k2sb1.tile([4, N1], F32)
                rhs1 = k2sb1.tile([4, N1], F32)
                vT_ps = k2ps1.tile([3, N1], F32)
                for g in range(2):
                    nc.tensor.transpose(
                        vT_ps[:, g * 128:(g + 1) * 128], v1g[:, g, 0:3], ident_sb
                    )
                nc.scalar.activation(lhsT1[0:3, :], vT_ps, ACTF.Copy)
                nc.sync.dma_start(lhsT1[3:4, :], ones_row[:, 0:N1])
                nc.vector.tensor_scalar_mul(rhs1[0:3, :], lhsT1[0:3, :], 2.0)
                v1sq = k2sb1.tile([128, 2, 3], F32)
                nc.vector.tensor_mul(v1sq, v1g[:, :, 0:3], v1g[:, :, 0:3])
                q1 = k2sb1.tile([128, 2], F32)
                nc.vector.tensor_reduce(q1, v1sq, axis=AXX, op=ALU.add)
                q1T_ps = k2ps1.tile([2, 128], F32)
                nc.tensor.transpose(q1T_ps, q1, ident_sb)
                q1T = k2sb1.tile([2, 128], F32)
                nc.scalar.activation(q1T, q1T_ps, ACTF.Copy, scale=-1.0)
                nc.sync.dma_start(
                    rhs1[3:4, :].rearrange("a (g p) -> a g p", p=128), q1T
                )
                for g in range(2):
                    m2_ps = k2ps.tile([128, N1], F32, tag="m2_ps")
                    nc.tensor.matmul(
                        m2_ps, lhsT1[:, g * 128:(g + 1) * 128], rhs1,
                        start=True, stop=True,
                    )
                    m2 = k2sb.tile([128, N1], F32, tag="m2")
                    nc.scalar.activation(m2, m2_ps, ACTF.Copy)
                    nc.vector.tensor_add(
                        m2[:, g * 128:(g + 1) * 128],
                        m2[:, g * 128:(g + 1) * 128], neg_i_sb,
                    )
                    for r in range(3):
                        rv2 = k2sb.tile([128, 8], F32, tag="rv2")
                        nc.vector.max(out=rv2, in_=m2)
                        nc.vector.max_index(
                            out=NI2[:, g, r * 8:(r + 1) * 8], in_max=rv2, in_values=m2
                        )
                        if r < 2:
                            nc.vector.match_replace(
                                out=m2, in_to_replace=rv2, in_values=m2, imm_value=NEG
                            )

                nc.vector.tensor_copy(NI232, NI2[:, :, 0:NB])
                nc.sync.dma_start(
                    _ap(TNI2, [[128, 128], [16384, 2], [1, 24]]),
                    NI2[:, :, :].bitcast(I16),
                )
                for g in range(2):
                    nc.sync.dma_start(
                        _ap(FC, [[1, 128], [128, NB], [1, 1]], off=g * 2560),
                        NI2[:, g:g + 1, 0:NB].bitcast(I16).rearrange("p a n -> p n a"),
                    )
                    nc.sync.dma_start(
                        _ap(FCT, [[160, 16], [1, 160], [1, 1]], off=g * 2560),
                        _ap(FC, [[1, 16], [16, 160], [1, 1]], off=g * 2560),
                    )
                nc.sync.dma_start(idxw2, _ap(FCT, [[160, 16], [2560, 2], [1, 160]]))

            # ---- Phase 6: feat2 + conv2 (layer 2) ----
            with (
                tc.tile_pool(name="l2_ps", bufs=2, space="PSUM") as l2ps,
                tc.tile_pool(name="l2_sb", bufs=1) as l2sb,
                tc.tile_pool(name="l2_sb2", bufs=2) as l2sb2,
            ):
                f2c = l2sb.tile([128, 2, C2], F32)
                for g in range(2):
                    f2_ps = l2ps.tile([128, 4, 512], F32, tag="big_ps", bufs=2)
                    for j in range(4):
                        nc.tensor.matmul(
                            f2_ps[:, j, :], f2T[:, g, :],
                            wb2_sb[:, j * 512:(j + 1) * 512],
                            start=True, stop=True,
                        )
                    f2flat = f2_ps.rearrange("p a b -> p (a b)")
                    nc.scalar.activation(f2c[:, g, :], f2flat[:, 0:C2], ACTF.Copy)
                    f2s = l2sb2.tile([128, SC2], F16, tag="f2s")
                    nc.scalar.activation(f2s, f2flat[:, C2:2048], ACTF.Copy)
                    nc.sync.dma_start(
                        _ap(T3, [[SC2, 128], [1, SC2]], off=g * 128 * SC2), f2s
                    )

                VT116 = l2sb.tile([16, N1], F32)
                nc.vector.memset(VT116, 0.0)
                nc.sync.dma_start(VT116[0:3, :], _ap(T0v1, [[1, 3], [64, N1]]))
                for g in range(2):
                    vjT2 = l2sb2.tile([16, NB * 128], F32, tag="vjT2", bufs=1)
                    nc.gpsimd.ap_gather(
                        vjT2, VT116, idxw2[:, g, :],
                        channels=16, num_elems=N1, d=1, num_idxs=NB * 128,
                    )
                    dispT2 = l2sb2.tile([3, NB, 128], F16, tag="dispT2", bufs=1)
                    nc.vector.tensor_sub(
                        dispT2,
                        vjT2[0:3, :].rearrange("d (n i) -> d n i", n=NB),
                        VT116[0:3, g * 128:(g + 1) * 128]
                        .rearrange("d (a i) -> d a i", a=1)
                        .to_broadcast([3, NB, 128]),
                    )

                    acc = l2sb2.tile([128, SC2], F16, tag="acc", bufs=1)
                    th2 = None
                    for h in range(2):
                        fg2 = l2sb2.tile([128, 10, SC2], F16, tag="fg2", bufs=2)
                        for j in range(10):
                            nn = h * 10 + j
                            nc.gpsimd.indirect_dma_start(
                                out=fg2[:, j, :], out_offset=None, in_=T3[:, :],
                                in_offset=IOA(ap=NI232[:, g, nn:nn + 1], axis=0),
                            )
                        th2 = l2sb2.tile([128, 10, SC2], F16, tag="th2", bufs=1)
                        for j in range(10):
                            nn = h * 10 + j
                            th2_ps = l2ps.tile([128, 4, 512], F32, tag="big_ps", bufs=2)
                            for u in range(4):
                                w = min(512, SC2 - u * 512)
                                nc.tensor.matmul(
                                    th2_ps[:, u, 0:w],
                                    dispT2[:, nn, :],
                                    w2d_sb[:, u * 512:u * 512 + w],
                                    start=True, stop=True,
                                )
                            nc.scalar.activation(
                                th2[:, j, :],
                                th2_ps.rearrange("p a b -> p (a b)")[:, 0:SC2],
                                ACTF.Relu,
                            )
                        nc.vector.tensor_mul(th2, th2, fg2)
                        nc.vector.tensor_tensor(
                            out=th2[:, 0:5, :], in0=th2[:, 0:5, :], in1=th2[:, 5:10, :],
                            op=ALU.max,
                        )
                        nc.vector.tensor_tensor(
                            out=th2[:, 0:2, :], in0=th2[:, 0:2, :], in1=th2[:, 2:4, :],
                            op=ALU.max,
                        )
                        nc.vector.tensor_tensor(
                            out=th2[:, 0:1, :], in0=th2[:, 0:1, :], in1=th2[:, 1:2, :],
                            op=ALU.max,
                        )
                        if h == 0:
                            nc.vector.tensor_tensor(
                                out=acc, in0=th2[:, 0, :], in1=th2[:, 4, :], op=ALU.max
                            )
                        else:
                            nc.vector.tensor_tensor(
                                out=th2[:, 0:1, :], in0=th2[:, 0:1, :],
                                in1=th2[:, 4:5, :], op=ALU.max,
                            )
                    a2t = l2sb2.tile([128, C2, SUP], F32, tag="a2t", bufs=1)
                    nc.vector.tensor_tensor(
                        out=a2t.rearrange("p c s -> p s c"),
                        in0=acc.rearrange("p (s c) -> p s c", s=SUP),
                        in1=th2[:, 0, :].rearrange("p (s c) -> p s c", s=SUP),
                        op=ALU.max,
                    )
                    fm2 = l2sb2.tile([128, C2], F32, tag="fm2", bufs=1)
                    nc.vector.tensor_reduce(fm2, a2t, axis=AXX, op=ALU.add)
                    nc.vector.tensor_add(fm2, fm2, f2c[:, g, :])
                    nc.vector.tensor_scalar_max(fm2, fm2, 0.0)
                    fm2h = l2sb2.tile([128, C2], F16, tag="fm2h", bufs=1)
                    nc.vector.tensor_copy(fm2h, fm2)
                    nc.sync.dma_start(
                        _ap(T4, [[C2, 128], [1, C2]], off=g * 128 * C2), fm2h
                    )

            # ---- Phase 7: pooling 2 + outputs ----
            with tc.tile_pool(name="p2_sb", bufs=1) as p2sb:
                NIs2 = p2sb.tile([16, 128], I16)
                nc.gpsimd.indirect_dma_start(
                    out=NIs2, out_offset=None, in_=TNI2[:, :],
                    in_offset=IOA(ap=s2pp_sb[:, 0:1], axis=0),
                )
                NIs232 = p2sb.tile([16, 16], I32)
                nc.vector.tensor_copy(NIs232, NIs2[:, 0:16])
                fmg2 = p2sb.tile([16, 16, C2], F16)
                for n in range(16):
                    nc.gpsimd.indirect_dma_start(
                        out=fmg2[:, n, :], out_offset=None, in_=T4[:, :],
                        in_offset=IOA(ap=NIs232[:, n:n + 1], axis=0),
                    )
                nc.vector.tensor_tensor(
                    out=fmg2[:, 0:8, :], in0=fmg2[:, 0:8, :], in1=fmg2[:, 8:16, :],
                    op=ALU.max,
                )
                nc.vector.tensor_tensor(
                    out=fmg2[:, 0:4, :], in0=fmg2[:, 0:4, :], in1=fmg2[:, 4:8, :],
                    op=ALU.max,
                )
                nc.vector.tensor_tensor(
                    out=fmg2[:, 0:2, :], in0=fmg2[:, 0:2, :], in1=fmg2[:, 2:4, :],
                    op=ALU.max,
                )
                nc.vector.tensor_tensor(
                    out=fmg2[:, 0:1, :], in0=fmg2[:, 0:1, :], in1=fmg2[:, 1:2, :],
                    op=ALU.max,
                )
                fm3f = p2sb.tile([16, C2], F32)
                nc.vector.tensor_copy(fm3f, fmg2[:, 0, :])
                nc.sync.dma_start(out_fm3[:, :], fm3f[0:N2, :])

                v2g = p2sb.tile([16, 64], F32)
                nc.gpsimd.indirect_dma_start(
                    out=v2g, out_offset=None, in_=T0v1[:, :],
                    in_offset=IOA(ap=s2pp_sb[:, 0:1], axis=0),
                )
                nc.sync.dma_start(out_v2[:, :], v2g[0:N2, 0:3])

    nc.compile()
    return nc


_NC_CACHE = None


def _get_nc():
    global _NC_CACHE
    if _NC_CACHE is None:
        _NC_CACHE = build_nc()
    return _NC_CACHE


def make_in_maps(inputs):
    v = np.asarray(inputs["vertices"], dtype=np.float32)
    w0_weights = np.asarray(inputs["w0_weights"], dtype=np.float32)
    w0_disp = np.asarray(inputs["w0_disp"], dtype=np.float32)
    w1_weights = np.asarray(inputs["w1_weights"], dtype=np.float32)
    w1_bias = np.asarray(inputs["w1_bias"], dtype=np.float32)
    w1_disp = np.asarray(inputs["w1_disp"], dtype=np.float32)
    w2_weights = np.asarray(inputs["w2_weights"], dtype=np.float32)
    w2_bias = np.asarray(inputs["w2_bias"], dtype=np.float32)
    w2_disp = np.asarray(inputs["w2_disp"], dtype=np.float32)

    shared = {
        "w01_disp": np.ascontiguousarray(
            np.concatenate([w0_disp, w1_disp], axis=1)
        ).astype(np.float16),
        "w2_disp": np.ascontiguousarray(w2_disp).astype(np.float16),
        "w0w": np.ascontiguousarray(
            np.tile(w0_weights.reshape(1, SUP * K0), (128, 1))
        ),
        "wb1": np.ascontiguousarray(
            np.concatenate([w1_weights, w1_bias[None, :]], axis=0)
        ),
        "wb2": np.ascontiguousarray(
            np.concatenate([w2_weights, w2_bias[None, :]], axis=0)
        ),
        "neg_i": (np.eye(128, dtype=np.float32) * NEG).astype(np.float32),
        "identity": np.eye(128, dtype=np.float32),
        "ones_row": np.ones((1, N), dtype=np.float32),
        "samp1_pp": np.ascontiguousarray(
            SAMPLE1.reshape(2, 128).T.astype(np.int32)
        ),
        "samp2_pp": SAMPLE2.reshape(16, 1).astype(np.int32),
    }
    in_maps = []
    for b in range(v.shape[0]):
        m = dict(shared)
        m["vertices"] = np.ascontiguousarray(v[b])
        in_maps.append(m)
    return in_maps


def kernel(**inputs):
    from concourse import bass_utils

    nc = _get_nc()
    in_maps = make_in_maps(inputs)
    res = bass_utils.run_bass_kernel_spmd(nc, in_maps, core_ids=list(range(BS)))
    v2 = np.stack([res.results[b]["out_v2"] for b in range(BS)])
    fm3 = np.stack([res.results[b]["out_fm3"] for b in range(BS)])
    return v2.astype(np.float32), fm3.astype(np.float32)
